# revision 7
# baseline (speedup 1.0000x reference)
"""Low-shelf biquad IIR filter as an exact FIR convolution on Trainium2.

The reference applies a Direct-Form-II-Transposed biquad (low-shelf,
fc=1000Hz, gain=+6dB, Q=0.7, fs=16kHz) independently to each of 32
waveforms of length 960000.  The filter poles have radius sqrt(a2) ~=
0.789, so the impulse response decays below f32 precision within ~110
samples.  Hence the IIR equals (to fp32 accuracy) a 256-tap FIR:

    y[128c + p] = sum_j h[p-j]      x[128c + j]      (H0, taps 0..127)
                + sum_j h[128+p-j]  x[128(c-1) + j]  (H1, taps 1..255)

which maps onto the PE array as two 128-contraction matmuls per block of
chunks, accumulated in PSUM.  Data is transposed into [within-chunk,
chunk] layout via PE transposes and transposed back after the conv.

Sharding: batch dim (32) split across 8 cores, 4 rows each.
"""

import os
import sys

import numpy as np

_REPO = "/opt/trn_rl_repo"
if _REPO not in sys.path:
    sys.path.insert(0, _REPO)

# ---------------------------------------------------------------- constants
SAMPLE_RATE = 16000.0
CENTER_FREQ = 1000.0
GAIN_DB = 6.0
Q_FACTOR = 0.7

B, T = 32, 960000
N_CORES = 8
ROWS = B // N_CORES          # 4 rows per core
L = 128                      # chunk length == FIR matmul contraction
C = T // L                   # 7500 chunks per row (exact)
TPG = 4                      # transpose tiles per conv group (N = 512)
NT = (C + L - 1) // L        # 59 transpose tiles per row (58 full + 76)
TAIL_W = C - (NT - 1) * L    # 76 chunks in the last transpose tile

H0_DTYPE = "float32"
H1_DTYPE = "float32"


def _low_shelf_coeffs():
    w0 = 2.0 * np.pi * CENTER_FREQ / SAMPLE_RATE
    A = 10.0 ** (GAIN_DB / 40.0)
    alpha = np.sin(w0) / (2.0 * Q_FACTOR)
    cosw = np.cos(w0)
    sqA = np.sqrt(A)
    b0 = A * ((A + 1.0) - (A - 1.0) * cosw + 2.0 * sqA * alpha)
    b1 = 2.0 * A * ((A - 1.0) - (A + 1.0) * cosw)
    b2 = A * ((A + 1.0) - (A - 1.0) * cosw - 2.0 * sqA * alpha)
    a0 = (A + 1.0) + (A - 1.0) * cosw + 2.0 * sqA * alpha
    a1 = -2.0 * ((A - 1.0) + (A + 1.0) * cosw)
    a2 = (A + 1.0) + (A - 1.0) * cosw - 2.0 * sqA * alpha
    return (np.float32(b0 / a0), np.float32(b1 / a0), np.float32(b2 / a0),
            np.float32(a1 / a0), np.float32(a2 / a0))


def _impulse_response(n=2 * L):
    """Impulse response of the biquad, simulated in float64 with the
    fp32-rounded coefficients (matching the reference's arithmetic)."""
    b0, b1, b2, a1, a2 = (float(v) for v in _low_shelf_coeffs())
    h = np.zeros(n, dtype=np.float64)
    s1 = s2 = 0.0
    for t in range(n):
        xn = 1.0 if t == 0 else 0.0
        y = b0 * xn + s1
        s1 = b1 * xn - a1 * y + s2
        s2 = b2 * xn - a2 * y
        h[t] = y
    return h


def _fir_matrices():
    """H0T[j, p] = h[p - j]; H1T[j, p] = h[128 + p - j] (lhsT layout)."""
    h = _impulse_response(2 * L)
    j = np.arange(L)[:, None]
    p = np.arange(L)[None, :]
    d0 = p - j
    h0t = np.where(d0 >= 0, h[np.clip(d0, 0, 2 * L - 1)], 0.0)
    d1 = L + p - j
    h1t = h[np.clip(d1, 0, 2 * L - 1)]
    return h0t.astype(np.float32), h1t.astype(np.float32)


# ---------------------------------------------------------------- device code
_CACHE = {}


def _build_nc(T_local=T, rows=ROWS):
    from concourse import bacc, tile, mybir

    C = T_local // L
    NT = (C + L - 1) // L
    TAIL_W = C - (NT - 1) * L

    f32 = mybir.dt.float32
    nc = bacc.Bacc("TRN2", target_bir_lowering=False, debug=False,
                   num_devices=N_CORES)

    x_d = nc.dram_tensor("x", [rows, T_local], f32, kind="ExternalInput")
    h0_d = nc.dram_tensor("h0", [L, L], f32, kind="ExternalInput")
    h1_d = nc.dram_tensor("h1", [L, L], f32, kind="ExternalInput")
    id_d = nc.dram_tensor("ident", [L, L], f32, kind="ExternalInput")
    y_d = nc.dram_tensor("y", [rows, T_local], f32, kind="ExternalOutput")

    # conv groups per row: (chunk0, n_chunks, tile indices)
    groups = []
    t0 = 0
    while t0 < NT:
        tiles = list(range(t0, min(t0 + TPG, NT)))
        c0 = t0 * L
        n = sum(TAIL_W if t == NT - 1 else L for t in tiles)
        groups.append((c0, n, tiles))
        t0 += TPG

    MAIN_BLKS = NT - 1          # 58 full transpose tiles covered by main DMA
    MAIN_ELEMS = MAIN_BLKS * L * L

    with tile.TileContext(nc) as tc:
        with (
            tc.tile_pool(name="const", bufs=1) as cpool,
            tc.tile_pool(name="xn", bufs=2) as xnpool,
            tc.tile_pool(name="xt", bufs=1) as xtpool,
            tc.tile_pool(name="ysb", bufs=4) as ypool,
            tc.tile_pool(name="yn", bufs=2) as ynpool,
            tc.tile_pool(name="pst", bufs=2, space="PSUM") as pstpool,
            tc.tile_pool(name="psy", bufs=2, space="PSUM") as psypool,
            tc.tile_pool(name="psn", bufs=2, space="PSUM") as psnpool,
        ):
            h0_sb = cpool.tile([L, L], f32, tag="h0")
            h1_sb = cpool.tile([L, L], f32, tag="h1")
            id_sb = cpool.tile([L, L], f32, tag="ident")
            nc.sync.dma_start(h0_sb[:], h0_d[:])
            nc.sync.dma_start(h1_sb[:], h1_d[:])
            nc.sync.dma_start(id_sb[:], id_d[:])

            for r in range(rows):
                # ---- load row r into chunk-tile layout:
                # xn[ci, blk*L + j] = x[r, (blk*L + ci)*L + j]
                xn = xnpool.tile([L, NT * L], f32, tag="xn")
                xn_main = xn[:, :MAIN_BLKS * L].rearrange(
                    "ci (blk j) -> ci blk j", j=L)
                src_main = x_d[r, :MAIN_ELEMS].rearrange(
                    "(blk ci j) -> ci blk j", ci=L, j=L)
                nc.sync.dma_start(xn_main, src_main)
                src_tail = x_d[r, MAIN_ELEMS:T_local].rearrange(
                    "(ci j) -> ci j", j=L)
                nc.sync.dma_start(xn[:TAIL_W, MAIN_BLKS * L:], src_tail)

                # ---- transposed layout [j, chunk]; col 0 = zero history
                xt = xtpool.tile([L, C + 1], f32, tag="xt")
                nc.gpsimd.memset(xt[:, 0:1], 0.0)

                yn = ynpool.tile([L, NT * L], f32, tag="yn")

                for (c0, n, tiles) in groups:
                    # transpose 4 input tiles into one PSUM bank
                    pst = pstpool.tile([L, TPG * L], f32, tag="pst")
                    for ti, t in enumerate(tiles):
                        w = TAIL_W if t == NT - 1 else L
                        nc.tensor.transpose(
                            pst[:, ti * L: ti * L + w],
                            xn[:w, t * L:(t + 1) * L],
                            id_sb[:w, :w],
                        )
                    nc.vector.tensor_copy(xt[:, 1 + c0: 1 + c0 + n],
                                          pst[:, :n])

                    # conv: y_c = H0 @ x_c + H1 @ x_{c-1}
                    psy = psypool.tile([L, TPG * L], f32, tag="psy")
                    nc.tensor.matmul(psy[:, :n], h0_sb[:],
                                     xt[:, 1 + c0: 1 + c0 + n],
                                     start=True, stop=False)
                    nc.tensor.matmul(psy[:, :n], h1_sb[:],
                                     xt[:, c0: c0 + n],
                                     start=False, stop=True)
                    ysb = ypool.tile([L, TPG * L], f32, tag="ysb")
                    nc.scalar.copy(ysb[:, :n], psy[:, :n])

                    # transpose back to natural layout
                    psn = psnpool.tile([L, TPG * L], f32, tag="psn")
                    for ti, t in enumerate(tiles):
                        w = TAIL_W if t == NT - 1 else L
                        nc.tensor.transpose(
                            psn[:w, ti * L: ti * L + L],
                            ysb[:, ti * L: ti * L + w],
                            id_sb[:],
                        )
                    has_tail = (tiles[-1] == NT - 1) and (TAIL_W < L)
                    full_cols = (len(tiles) - (1 if has_tail else 0)) * L
                    if full_cols:
                        nc.vector.tensor_copy(yn[:, c0: c0 + full_cols],
                                              psn[:, :full_cols])
                    if has_tail:
                        s = full_cols
                        nc.vector.tensor_copy(
                            yn[:TAIL_W, c0 + s: c0 + s + L],
                            psn[:TAIL_W, s: s + L])

                # ---- store row r
                yn_main = yn[:, :MAIN_BLKS * L].rearrange(
                    "ci (blk j) -> ci blk j", j=L)
                dst_main = y_d[r, :MAIN_ELEMS].rearrange(
                    "(blk ci j) -> ci blk j", ci=L, j=L)
                nc.sync.dma_start(dst_main, yn_main)
                dst_tail = y_d[r, MAIN_ELEMS:T_local].rearrange(
                    "(ci j) -> ci j", j=L)
                nc.sync.dma_start(dst_tail, yn[:TAIL_W, MAIN_BLKS * L:])

    nc.compile()
    return nc


def _get_nc():
    if "nc" not in _CACHE:
        _CACHE["nc"] = _build_nc()
    return _CACHE["nc"]


def _in_maps(x_full):
    h0t, h1t = _fir_matrices()
    ident = np.eye(L, dtype=np.float32)
    maps = []
    for core in range(N_CORES):
        rows = x_full[core * ROWS:(core + 1) * ROWS]
        maps.append({
            "x": np.ascontiguousarray(rows, dtype=np.float32),
            "h0": h0t, "h1": h1t, "ident": ident,
        })
    return maps


def _run_device(x_full, trace=False):
    from concourse.bass_utils import run_bass_kernel_spmd

    nc = _get_nc()
    res = run_bass_kernel_spmd(nc, _in_maps(x_full), list(range(N_CORES)),
                               trace=trace)
    y = np.concatenate([res.results[i]["y"] for i in range(N_CORES)], axis=0)
    return y, res


def _timed_run(x_full, iters=5):
    """Run via a cached jitted sharded executable; time device-only execution
    (inputs pre-staged on device, outputs left on device during timing).
    Returns (y_full, [per-call seconds])."""
    import time

    import jax
    import jax.numpy as jnp  # noqa: F401
    from jax.sharding import Mesh, NamedSharding, PartitionSpec
    from jax.experimental.shard_map import shard_map
    from concourse import mybir
    from concourse.bass2jax import (_bass_exec_p, install_neuronx_cc_hook,
                                    partition_id_tensor)

    nc = _get_nc()
    install_neuronx_cc_hook()

    partition_name = (nc.partition_id_tensor.name
                      if nc.partition_id_tensor else None)
    in_names, out_names, out_avals, zero_outs = [], [], [], []
    for alloc in nc.m.functions[0].allocations:
        if not isinstance(alloc, mybir.MemoryLocationSet):
            continue
        name = alloc.memorylocations[0].name
        if alloc.kind == "ExternalInput":
            if name != partition_name:
                in_names.append(name)
        elif alloc.kind == "ExternalOutput":
            shape = tuple(alloc.tensor_shape)
            dtype = mybir.dt.np(alloc.dtype)
            out_names.append(name)
            out_avals.append(jax.core.ShapedArray(shape, dtype))
            zero_outs.append(np.zeros(shape, dtype))
    n_params = len(in_names)
    n_outs = len(out_avals)
    all_in_names = in_names + out_names
    if partition_name is not None:
        all_in_names.append(partition_name)

    def _body(*args):
        operands = list(args)
        if partition_name is not None:
            operands.append(partition_id_tensor())
        return tuple(_bass_exec_p.bind(
            *operands,
            out_avals=tuple(out_avals),
            in_names=tuple(all_in_names),
            out_names=tuple(out_names),
            lowering_input_output_aliases=(),
            sim_require_finite=True,
            sim_require_nnan=True,
            nc=nc,
        ))

    devices = jax.devices()[:N_CORES]
    mesh = Mesh(np.asarray(devices), ("core",))
    spec = PartitionSpec("core")
    shard = NamedSharding(mesh, spec)
    donate = tuple(range(n_params, n_params + n_outs))
    fn = jax.jit(
        shard_map(_body, mesh=mesh, in_specs=(spec,) * (n_params + n_outs),
                  out_specs=(spec,) * n_outs, check_rep=False),
        donate_argnums=donate, keep_unused=True,
    )

    maps = _in_maps(x_full)
    concat_in = [np.concatenate([maps[c][nm] for c in range(N_CORES)], axis=0)
                 for nm in in_names]
    dev_in = [jax.device_put(a, shard) for a in concat_in]
    concat_zero_shapes = [(N_CORES * z.shape[0], *z.shape[1:])
                          for z in zero_outs]

    times = []
    outs = None
    for _ in range(iters):
        dev_zeros = [jax.device_put(np.zeros(s, z.dtype), shard)
                     for s, z in zip(concat_zero_shapes, zero_outs)]
        for a in dev_zeros:
            a.block_until_ready()
        t0 = time.perf_counter()
        outs = fn(*dev_in, *dev_zeros)
        for o in outs:
            o.block_until_ready()
        times.append(time.perf_counter() - t0)

    yi = out_names.index("y")
    y = np.asarray(outs[yi]).reshape(N_CORES, ROWS, T).reshape(B, T)
    return y, times


def kernel(input_tensor, input_lengths):
    x = np.asarray(input_tensor, dtype=np.float32)
    assert x.shape == (B, T), x.shape
    y, _ = _run_device(x, trace=False)
    return y, np.asarray(input_lengths)
